# revision 1
# baseline (speedup 1.0000x reference)
"""Causal self-attention kernel for Trainium2, 8 NeuronCores.

Sharding: DP4 x TP2. Core c = 2*b + g handles batch b (2048 tokens) and
head-group g (8 of 16 heads). Per core:
  - x arrives pre-cast to bf16 and is transposed by the DMA xbar
    (d_model onto partitions), no PE involvement,
  - QKV matmuls in bf16: Q,K dim-major ([head_dim, tokens]), V token-major
    padded to 128 columns with a ones column (softmax denominator for free),
  - attention per head pair: scores^T = K_h^T-tile @ Q_h in [k, q] layout
    with both heads' QK matmuls in different PE row groups (concurrent),
    one wide exp on ACT (1/sqrt(64) folded into its scale) into bf16 probs,
    causal handling by skipping fully-masked tiles, memset on fully-masked
    column ranges and a 0/1 mask multiply on the 128-wide diagonal band,
  - normalization via reciprocal_approx_fast + gpsimd partition_broadcast,
  - fp32r projection with the w_proj row shard (token-major output),
  - chunked pairwise AllReduce (cores 2b, 2b+1) overlapped with later tiles.

QKV work for token tile n+1 is emitted interleaved with attention for tile n
so the PE always has independent matmuls while ACT drains the exps.

Everything (shapes, sharding) is hardcoded for
x: [4, 2048, 1024], w_qkv: [1024, 3072], w_proj: [1024, 1024], f32.
"""

import ml_dtypes
import numpy as np

import concourse.bacc as bacc
import concourse.mybir as mybir
import concourse.tile as tile
from concourse.tile import add_dep_helper
from concourse.bass_utils import run_bass_kernel_spmd

F32 = mybir.dt.float32
F32R = mybir.dt.float32r
BF16 = mybir.dt.bfloat16

S = 2048  # tokens per core (one batch element)
D = 1024  # d_model
HL = 8  # heads per core (local)
HD = 64  # head dim
GD = HL * HD  # 512, head-group dim
NQT = S // 512  # 4 q-tiles of 512
NDM = D // 128  # 8 d_model chunks
NTOK = S // 128  # 16 token tiles of 128

_NC_CACHE = {}


def _qkv_units(nc, P, n):
    """QKV matmul chains for token tile n, as separately emittable units."""
    units = []

    def qk_chain(m):
        def emit():
            ps = P.b1_ps.tile([128, 512], F32, tag="b1", name="qkps")
            for k in range(NDM):
                nc.tensor.matmul(
                    ps,
                    P.w_sb[:, k, m * 128 : (m + 1) * 128],
                    P.xT[:, k, n * 512 : (n + 1) * 512],
                    start=(k == 0),
                    stop=(k == NDM - 1),
                )
            nc.vector.tensor_copy(
                out=P.qkT[:, m, n * 512 : (n + 1) * 512], in_=ps
            )

        return emit

    def v_chain(t4):
        def emit():
            t = n * 4 + t4
            ps = P.b1_ps.tile([128, 512], F32, tag="b1", name="vps")
            for k in range(NDM):
                nc.tensor.matmul(
                    ps,
                    P.xT[:, k, t * 128 : (t + 1) * 128],
                    P.w_sb[:, k, 2 * GD : 3 * GD],
                    start=(k == 0),
                    stop=(k == NDM - 1),
                )
            nc.vector.tensor_copy(
                out=P.v_sb[:, t, :, 0:HD],
                in_=ps.rearrange("p (h d) -> p h d", h=HL),
            )

        return emit

    for m in range(2 * GD // 128):
        units.append(qk_chain(m))
    for t4 in range(4):
        units.append(v_chain(t4))
    return units


def _attn_units(nc, P, j):
    """Attention units for q-tile j: per head pair, c-groups + epilogue."""
    units = []
    for hp in range(HL // 2):
        yps = {}

        def alloc(hp=hp, yps=yps):
            for hi in range(2):
                yps[hi] = P.y_ps.tile(
                    [128, 512], F32, tag=f"yps{hi}", name=f"yps{hi}", bufs=1
                )

        units.append(alloc)

        def cgroup(c, hp=hp, yps=yps):
            def emit():
                d = c - 4 * j  # >= 0 on the diagonal band
                off = max(d, 0) * 128  # columns below off are fully masked
                sps2 = P.attn_ps.tile(
                    [128, 2, 512], F32, tag="sps2", name="sps2"
                )
                for hi in range(2):
                    h = 2 * hp + hi
                    po = (h % 2) * 64
                    nc.tensor.matmul(
                        sps2[:, hi, off:512],
                        P.qkT[po : po + 64, 4 + h // 2, c * 128 : (c + 1) * 128],
                        P.qkT[po : po + 64, h // 2, j * 512 + off : (j + 1) * 512],
                        start=True,
                        stop=True,
                    )
                probs2 = P.probs_p.tile(
                    [128, 2, 512], BF16, tag="probs", name="probs"
                )
                if off:
                    nc.vector.memset(probs2[:, :, 0:off], 0.0)
                nc.scalar.activation(
                    out=probs2[:, :, off:512],
                    in_=sps2[:, :, off:512],
                    func=mybir.ActivationFunctionType.Exp,
                    scale=0.125,
                )
                if d >= 0:
                    for hi in range(2):
                        nc.vector.tensor_mul(
                            probs2[:, hi, off : off + 128],
                            probs2[:, hi, off : off + 128],
                            P.mask_sb,
                        )
                for hi in range(2):
                    h = 2 * hp + hi
                    nc.tensor.matmul(
                        yps[hi],
                        P.v_sb[:, c, h, :],
                        probs2[:, hi, :],
                        start=(c == 0),
                        stop=(c == 4 * j + 3),
                    )

            return emit

        for c in range(4 * j + 4):
            units.append(cgroup(c))

        def epilogue(hp=hp, yps=yps):
            # ones-row out of PSUM, fast reciprocal, partition broadcast,
            # scale y into dim-major yT
            for hi in range(2):
                h = 2 * hp + hi
                po = (h % 2) * 64
                den = P.den_p.tile([1, 512], F32, tag="den", name="den")
                nc.scalar.activation(
                    out=den,
                    in_=yps[hi][HD : HD + 1, :],
                    func=mybir.ActivationFunctionType.Copy,
                )
                nc.vector.reciprocal_approx_fast(out=den, in_=den)
                denb = P.den_p.tile([HD, 512], F32, tag="denb", name="denb")
                nc.gpsimd.partition_broadcast(denb, den)
                nc.vector.tensor_mul(
                    P.yT[po : po + 64, h // 2, j * 512 : (j + 1) * 512],
                    yps[hi][0:HD, :],
                    denb,
                )

        units.append(epilogue)
    return units


def _proj_chunk(nc, P, j):
    """Projection for the 4 token tiles of q-tile j (token-major output)."""
    for mt in range(4 * j, 4 * j + 4):
        osb = P.out_p.tile([128, D], F32, tag="osb", name="osb")
        for nh in range(2):
            ps = P.b1_ps.tile([128, 512], F32, tag="b1", name="ops")
            for kk in range(GD // 128):
                nc.tensor.matmul(
                    ps,
                    P.yT[:, kk, mt * 128 : (mt + 1) * 128],
                    P.wp_sb[:, kk, nh * 512 : (nh + 1) * 512],
                    start=(kk == 0),
                    stop=(kk == GD // 128 - 1),
                )
            nc.vector.tensor_copy(out=osb[:, nh * 512 : (nh + 1) * 512], in_=ps)
        nc.sync.dma_start(out=P.cc_in[mt * 128 : (mt + 1) * 128, :], in_=osb)


def _ar_chunk(nc, P, j):
    """AllReduce + final output DMA for q-tile j's 512 token rows."""
    lo, hi = j * 512, (j + 1) * 512
    nc.gpsimd.collective_compute(
        "AllReduce",
        mybir.AluOpType.add,
        replica_groups=[[0, 1], [2, 3], [4, 5], [6, 7]],
        ins=[P.cc_in[lo:hi, :].opt()],
        outs=[P.cc_out[lo:hi, :].opt()],
    )
    for mt in range(4 * j, 4 * j + 4):
        nc.sync.dma_start(
            out=P.out[mt * 128 : (mt + 1) * 128, :],
            in_=P.cc_out[mt * 128 : (mt + 1) * 128, :],
        )


class _Ctx:
    pass


def _build_nc():
    nc = bacc.Bacc(None, num_devices=8)
    P = _Ctx()

    xb16 = nc.dram_tensor("xb16", [S, D], BF16, kind="ExternalInput").ap()
    wqkv = nc.dram_tensor("wqkv", [D, 3 * GD], BF16, kind="ExternalInput").ap()
    wproj = nc.dram_tensor("wproj", [GD, D], F32, kind="ExternalInput").ap()
    masks = nc.dram_tensor("masks", [128, 128], BF16, kind="ExternalInput").ap()
    P.out = nc.dram_tensor("out", [S, D], F32, kind="ExternalOutput").ap()

    with tile.TileContext(nc) as tc:
        with (
            tc.tile_pool(name="const", bufs=1) as const,
            tc.tile_pool(name="w_p", bufs=1) as w_p,
            tc.tile_pool(name="big_p", bufs=1) as big_p,
            tc.tile_pool(name="probs_p", bufs=6) as probs_p,
            tc.tile_pool(name="den_p", bufs=2) as den_p,
            tc.tile_pool(name="out_p", bufs=2) as out_p,
            tc.tile_pool(name="b1_ps", bufs=2, space="PSUM") as b1_ps,
            tc.tile_pool(name="attn_ps", bufs=2, space="PSUM") as attn_ps,
            tc.tile_pool(name="y_ps", bufs=1, space="PSUM") as y_ps,
            tc.tile_pool(name="dram", bufs=1, space="DRAM") as dram,
        ):
            P.probs_p, P.den_p, P.out_p = probs_p, den_p, out_p
            P.b1_ps, P.attn_ps, P.y_ps = b1_ps, attn_ps, y_ps

            # DMA xbar transpose first: xT[p, e, t] = x[t, e*128 + p].
            # The xbar transpose silently corrupts data when plain DMAs run
            # concurrently, so every other startup DMA gets an explicit
            # dependency edge on all transposes.
            P.xT = big_p.tile([128, NDM, S], BF16, name="xT")
            tr_insts = []
            for t in range(NTOK):
                tr_insts.append(
                    nc.sync.dma_start_transpose(
                        out=P.xT[:, :, t * 128 : (t + 1) * 128],
                        in_=xb16[t * 128 : (t + 1) * 128, :],
                    )
                )
            plain = []
            P.mask_sb = const.tile([128, 128], BF16, name="mask_sb")
            plain.append(nc.sync.dma_start(out=P.mask_sb, in_=masks))

            P.w_sb = w_p.tile([128, NDM, 3 * GD], BF16, name="w_sb")
            for k in range(NDM):
                plain.append(
                    nc.sync.dma_start(
                        out=P.w_sb[:, k, :], in_=wqkv[k * 128 : (k + 1) * 128, :]
                    )
                )
            P.wp_sb = w_p.tile([128, GD // 128, D], F32R, name="wp_sb")
            for kk in range(GD // 128):
                plain.append(
                    nc.sync.dma_start(
                        out=P.wp_sb[:, kk, :],
                        in_=wproj[kk * 128 : (kk + 1) * 128, :].bitcast(F32R),
                    )
                )
            for p_ in plain:
                for ti in tr_insts:
                    add_dep_helper(
                        p_.ins, ti.ins, sync=True,
                        reason="xbar transpose isolation",
                    )
            P.qkT = big_p.tile([128, 2 * GD // 128, S], BF16, name="qkT")
            P.v_sb = big_p.tile([128, NTOK, HL, 128], BF16, name="v_sb")
            nc.vector.memset(P.v_sb[:, :, :, HD:128], 0.0)
            nc.vector.memset(P.v_sb[:, :, :, HD : HD + 1], 1.0)
            P.yT = big_p.tile([128, GD // 128, S], F32R, name="yT")

            P.cc_in = dram.tile([S, D], F32, name="cc_in")
            P.cc_out = dram.tile([S, D], F32, name="cc_out")

            ZIPPER = False
            for u in _qkv_units(nc, P, 0):
                u()
            for n in range(NQT):
                a_units = _attn_units(nc, P, n)
                q_units = _qkv_units(nc, P, n + 1) if n + 1 < NQT else []
                qi = 0
                for i, u in enumerate(a_units):
                    u()
                    while ZIPPER and qi < len(q_units) and qi * len(
                        a_units
                    ) < (i + 1) * len(q_units):
                        q_units[qi]()
                        qi += 1
                for u in q_units[qi:]:
                    u()
                _proj_chunk(nc, P, n)
                _ar_chunk(nc, P, n)

    nc.compile()
    return nc


def _host_consts():
    ki = np.arange(128)[:, None]
    qj = np.arange(128)[None, :]
    masks = (qj >= ki).astype(ml_dtypes.bfloat16)  # [128, 128] diagonal band
    return masks


def _in_maps(x, w_qkv, w_proj):
    masks = _host_consts()
    maps = []
    for c in range(8):
        b, g = c // 2, c % 2
        wq = w_qkv[:, g * GD : (g + 1) * GD]
        wk = w_qkv[:, D + g * GD : D + (g + 1) * GD]
        wv = w_qkv[:, 2 * D + g * GD : 2 * D + (g + 1) * GD]
        maps.append(
            {
                "xb16": np.ascontiguousarray(x[b]).astype(ml_dtypes.bfloat16),
                "wqkv": np.ascontiguousarray(
                    np.concatenate([wq, wk, wv], axis=1)
                ).astype(ml_dtypes.bfloat16),
                "wproj": np.ascontiguousarray(w_proj[g * GD : (g + 1) * GD, :]),
                "masks": masks,
            }
        )
    return maps


def kernel(x, w_qkv, w_proj):
    x = np.ascontiguousarray(x, dtype=np.float32)
    w_qkv = np.ascontiguousarray(w_qkv, dtype=np.float32)
    w_proj = np.ascontiguousarray(w_proj, dtype=np.float32)
    if "nc" not in _NC_CACHE:
        _NC_CACHE["nc"] = _build_nc()
    nc = _NC_CACHE["nc"]
    r = run_bass_kernel_spmd(nc, _in_maps(x, w_qkv, w_proj), list(range(8)))
    return np.stack([r.results[2 * b]["out"] for b in range(4)], axis=0)



# revision 5
# speedup vs baseline: 1.1849x; 1.1849x over previous
"""Causal self-attention kernel for Trainium2, 8 NeuronCores.

Sharding: DP4 x TP2. Core c = 2*b + g handles batch b (2048 tokens) and
head-group g (8 of 16 heads). Per core:
  - x arrives pre-transposed AND pre-cast to bf16 on the host (d_model on
    partitions), so startup is plain parallel DMA (no xbar transposes),
  - QKV matmuls in bf16: Q,K dim-major ([head_dim, tokens]), V token-major
    65 wide (64 dims + a ones column -> softmax denominator for free),
  - attention per head pair: scores^T = K_h^T-tile @ Q_h in [k, q] layout
    with both heads' QK matmuls in different PE row groups (concurrent),
    one wide exp on ACT (1/sqrt(64) folded into its scale) into bf16 probs,
    causal handling by skipping fully-masked tiles, sub-tile column ranges
    on the diagonal band (scores, exp AND att@V all restricted to off:512),
    a 0/1 mask multiply on the 128-wide diagonal band,
  - the attention inner loop is software-pipelined (scores for c-tile n+1
    are emitted before att@V for tile n) so the in-order PE queue never
    blocks on the ACT exp,
  - QKV for token tile n+1 and projection partials for token tile n-1 are
    zipped between attention units of tile n to keep the PE warm while ACT
    drains the exps,
  - bf16 projection partials are summed across the core pair with chunked
    ReduceScatter (256 global rows per chunk, pipelined behind attention)
    instead of an fp32 AllReduce at the end: 4x less traffic, each core
    only receives + upcasts its own 128-row shard per chunk, and the last
    chunk's exposed tail is tiny,
  - host assembles the 4x2048x1024 output from the 8 interleaved shards.

Everything (shapes, sharding) is hardcoded for
x: [4, 2048, 1024], w_qkv: [1024, 3072], w_proj: [1024, 1024], f32.
"""

import ml_dtypes
import numpy as np

import concourse.bacc as bacc
import concourse.mybir as mybir
import concourse.tile as tile
from concourse.bass_utils import run_bass_kernel_spmd

F32 = mybir.dt.float32
BF16 = mybir.dt.bfloat16

S = 2048  # tokens per core (one batch element)
D = 1024  # d_model
HL = 8  # heads per core (local)
HD = 64  # head dim
GD = HL * HD  # 512, head-group dim
VW = HD + 1  # V row width: 64 dims + ones column (denominator)
NQT = S // 512  # 4 q-tiles of 512
NDM = D // 128  # 8 d_model chunks
NTOK = S // 128  # 16 token tiles of 128
NCH = 8  # ReduceScatter chunks (256 global rows -> 128 owned rows each)
RG = [[0, 1], [2, 3], [4, 5], [6, 7]]

_NC_CACHE = {}


def _qkv_units(nc, P, n):
    """QKV matmul chains for token tile n, as separately emittable units."""
    units = []

    def qk_chain(m):
        def emit():
            ps = P.b1_ps.tile([128, 512], F32, tag="b1", name="qkps")
            for k in range(NDM):
                nc.tensor.matmul(
                    ps,
                    P.w_sb[:, k, m * 128 : (m + 1) * 128],
                    P.xT[:, k, n * 512 : (n + 1) * 512],
                    start=(k == 0),
                    stop=(k == NDM - 1),
                )
            nc.vector.tensor_copy(
                out=P.qkT[:, m, n * 512 : (n + 1) * 512], in_=ps
            )

        return emit

    def v_chain(t4):
        def emit():
            t = n * 4 + t4
            ps = P.b1_ps.tile([128, 512], F32, tag="b1", name="vps")
            for k in range(NDM):
                nc.tensor.matmul(
                    ps,
                    P.xT[:, k, t * 128 : (t + 1) * 128],
                    P.w_sb[:, k, 2 * GD : 3 * GD],
                    start=(k == 0),
                    stop=(k == NDM - 1),
                )
            nc.vector.tensor_copy(
                out=P.v_sb[:, t, :, 0:HD],
                in_=ps.rearrange("p (h d) -> p h d", h=HL),
            )

        return emit

    for m in range(2 * GD // 128):
        units.append(qk_chain(m))
    for t4 in range(4):
        units.append(v_chain(t4))
    return units


def _attn_units(nc, P, j):
    """Attention units for q-tile j, software-pipelined per head pair."""
    units = []
    ncol = 4 * j + 4
    for hp in range(HL // 2):
        state = {}

        def alloc(state=state):
            state["yps"] = P.y_ps.tile(
                [128, 2, 512], F32, tag="yps", name="yps", bufs=1
            )

        def sc(c, hp=hp, state=state):
            def emit():
                d = c - 4 * j  # >= 0 on the diagonal band
                off = max(d, 0) * 128  # columns below off are fully masked
                sps2 = P.attn_ps.tile(
                    [128, 2, 512], F32, tag="sps2", name="sps2"
                )
                for hi in range(2):
                    h = 2 * hp + hi
                    po = (h % 2) * 64
                    nc.tensor.matmul(
                        sps2[:, hi, off:512],
                        P.qkT[po : po + 64, 4 + h // 2, c * 128 : (c + 1) * 128],
                        P.qkT[po : po + 64, h // 2, j * 512 + off : (j + 1) * 512],
                        start=True,
                        stop=True,
                    )
                probs2 = P.probs_p.tile(
                    [128, 2, 512], BF16, tag="probs", name="probs"
                )
                nc.scalar.activation(
                    out=probs2[:, :, off:512],
                    in_=sps2[:, :, off:512],
                    func=mybir.ActivationFunctionType.Exp,
                    scale=0.125,
                )
                if d >= 0:
                    for hi in range(2):
                        nc.vector.tensor_mul(
                            probs2[:, hi, off : off + 128],
                            probs2[:, hi, off : off + 128],
                            P.mask_sb,
                        )
                state[c] = (probs2, off)

            return emit

        def av(c, hp=hp, state=state):
            def emit():
                probs2, off = state.pop(c)
                yps = state["yps"]
                for hi in range(2):
                    h = 2 * hp + hi
                    nc.tensor.matmul(
                        yps[0:VW, hi, off:512],
                        P.v_sb[:, c, h, :],
                        probs2[:, hi, off:512],
                        start=(c == 0),
                        stop=(c == ncol - 1),
                    )

            return emit

        def epilogue(hp=hp, state=state):
            # ones-row out of PSUM (ACT can shift partitions), fast
            # reciprocal, partition broadcast, scale y into dim-major yT
            yps = state["yps"]
            den2 = P.den_p.tile([1, 2, 512], F32, tag="den", name="den")
            nc.scalar.activation(
                out=den2,
                in_=yps[HD : HD + 1, :, :],
                func=mybir.ActivationFunctionType.Copy,
            )
            nc.vector.reciprocal_approx_fast(out=den2, in_=den2)
            denb = P.den_p.tile([HD, 2, 512], F32, tag="denb", name="denb")
            nc.gpsimd.partition_broadcast(denb, den2)
            for hi in range(2):
                h = 2 * hp + hi
                po = (h % 2) * 64
                nc.vector.tensor_mul(
                    P.yT[po : po + 64, h // 2, j * 512 : (j + 1) * 512],
                    yps[0:HD, hi, :],
                    denb[:, hi, :],
                )

        units.append(alloc)
        units.append(sc(0))
        for c in range(1, ncol):
            units.append(sc(c))
            units.append(av(c - 1))
        units.append(av(ncol - 1))
        units.append(epilogue)
    return units


def _proj_units(nc, P, j):
    """bf16 projection partials for q-tile j -> cc_in rows, 2 units/tile."""
    units = []
    for mt in range(4 * j, 4 * j + 4):
        state = {}

        def half(nh, mt=mt, state=state):
            def emit():
                if nh == 0:
                    state["osb"] = P.out_p.tile(
                        [128, D], BF16, tag="osb", name="osb"
                    )
                osb = state["osb"]
                ps = P.b1_ps.tile([128, 512], F32, tag="b1", name="ops")
                for kk in range(GD // 128):
                    nc.tensor.matmul(
                        ps,
                        P.yT[:, kk, mt * 128 : (mt + 1) * 128],
                        P.wp_sb[:, kk, nh * 512 : (nh + 1) * 512],
                        start=(kk == 0),
                        stop=(kk == GD // 128 - 1),
                    )
                nc.vector.tensor_copy(
                    out=osb[:, nh * 512 : (nh + 1) * 512], in_=ps
                )
                if nh == 1:
                    nc.sync.dma_start(
                        out=P.cc_in[mt * 128 : (mt + 1) * 128, :], in_=osb
                    )

            return emit

        units.append(half(0))
        units.append(half(1))
    return units


def _rs_unit(nc, P, k):
    """ReduceScatter of 256 global rows; my 128-row shard -> cc_red."""

    def emit():
        nc.gpsimd.collective_compute(
            "ReduceScatter",
            mybir.AluOpType.add,
            replica_groups=RG,
            ins=[P.cc_in[k * 256 : (k + 1) * 256, :].opt()],
            outs=[P.cc_red[k * 128 : (k + 1) * 128, :].opt()],
        )

    return emit


def _cast_unit(nc, P, k):
    """Upcast my 128-row bf16 shard of chunk k to fp32 and DMA it out."""

    def emit():
        ci = P.cast_p.tile([128, D], BF16, tag="ci", name="ci")
        nc.sync.dma_start(out=ci, in_=P.cc_red[k * 128 : (k + 1) * 128, :])
        co = P.cast_p.tile([128, D], F32, tag="co", name="co")
        nc.vector.tensor_copy(out=co, in_=ci)
        nc.sync.dma_start(out=P.out[k * 128 : (k + 1) * 128, :], in_=co)

    return emit


def _zip(a_units, fill_units):
    """Spread fill_units evenly between a_units (fills go to the PE's idle
    slots while ACT drains the exps)."""
    na, nf = len(a_units), len(fill_units)
    if nf == 0:
        return list(a_units)
    pos = [int(na * (f + 1) / (nf + 1)) for f in range(nf)]
    out = []
    fi = 0
    for i, u in enumerate(a_units):
        out.append(u)
        while fi < nf and pos[fi] <= i:
            out.append(fill_units[fi])
            fi += 1
    out.extend(fill_units[fi:])
    return out


class _Ctx:
    pass


def _build_nc():
    nc = bacc.Bacc(None, num_devices=8)
    P = _Ctx()

    xTd = nc.dram_tensor("xT", [D, S], BF16, kind="ExternalInput").ap()
    wqkv = nc.dram_tensor("wqkv", [D, 3 * GD], BF16, kind="ExternalInput").ap()
    wproj = nc.dram_tensor("wproj", [GD, D], BF16, kind="ExternalInput").ap()
    masks = nc.dram_tensor("masks", [128, 128], BF16, kind="ExternalInput").ap()
    P.out = nc.dram_tensor("out", [NCH * 128, D], F32, kind="ExternalOutput").ap()

    with tile.TileContext(nc) as tc:
        with (
            tc.tile_pool(name="const", bufs=1) as const,
            tc.tile_pool(name="w_p", bufs=1) as w_p,
            tc.tile_pool(name="big_p", bufs=1) as big_p,
            tc.tile_pool(name="probs_p", bufs=6) as probs_p,
            tc.tile_pool(name="den_p", bufs=2) as den_p,
            tc.tile_pool(name="out_p", bufs=2) as out_p,
            tc.tile_pool(name="cast_p", bufs=2) as cast_p,
            tc.tile_pool(name="b1_ps", bufs=2, space="PSUM") as b1_ps,
            tc.tile_pool(name="attn_ps", bufs=2, space="PSUM") as attn_ps,
            tc.tile_pool(name="y_ps", bufs=1, space="PSUM") as y_ps,
            tc.tile_pool(name="dram", bufs=1, space="DRAM") as dram,
        ):
            P.probs_p, P.den_p, P.out_p, P.cast_p = probs_p, den_p, out_p, cast_p
            P.b1_ps, P.attn_ps, P.y_ps = b1_ps, attn_ps, y_ps

            # Startup: plain DMAs only, all overlappable. Weights + the
            # first QKV tile's x slices first so the PE can start ASAP.
            P.w_sb = w_p.tile([128, NDM, 3 * GD], BF16, name="w_sb")
            for k in range(NDM):
                nc.sync.dma_start(
                    out=P.w_sb[:, k, :], in_=wqkv[k * 128 : (k + 1) * 128, :]
                )
            P.xT = big_p.tile([128, NDM, S], BF16, name="xT")
            for k in range(NDM):
                nc.sync.dma_start(
                    out=P.xT[:, k, 0:512], in_=xTd[k * 128 : (k + 1) * 128, 0:512]
                )
            P.mask_sb = const.tile([128, 128], BF16, name="mask_sb")
            nc.sync.dma_start(out=P.mask_sb, in_=masks)
            # preload the exp table while DMAs run
            aw = const.tile([1, 2], F32, name="actwarm")
            nc.vector.memset(aw, 0.0)
            nc.scalar.activation(
                out=aw, in_=aw, func=mybir.ActivationFunctionType.Exp
            )
            P.wp_sb = w_p.tile([128, GD // 128, D], BF16, name="wp_sb")
            for kk in range(GD // 128):
                nc.sync.dma_start(
                    out=P.wp_sb[:, kk, :], in_=wproj[kk * 128 : (kk + 1) * 128, :]
                )
            for k in range(NDM):
                nc.sync.dma_start(
                    out=P.xT[:, k, 512:S], in_=xTd[k * 128 : (k + 1) * 128, 512:S]
                )

            P.qkT = big_p.tile([128, 2 * GD // 128, S], BF16, name="qkT")
            P.v_sb = big_p.tile([128, NTOK, HL, VW], BF16, name="v_sb")
            nc.vector.memset(P.v_sb[:, :, :, HD : HD + 1], 1.0)
            P.yT = big_p.tile([128, GD // 128, S], BF16, name="yT")

            P.cc_in = dram.tile([S, D], BF16, name="cc_in")
            P.cc_red = dram.tile([NCH * 128, D], BF16, name="cc_red")

            for u in _qkv_units(nc, P, 0):
                u()

            for j in range(NQT):
                a_units = _attn_units(nc, P, j)
                fills = []
                if j + 1 < NQT:
                    fills += _qkv_units(nc, P, j + 1)
                if j >= 1:
                    pu = _proj_units(nc, P, j - 1)
                    fills += pu[0:4] + [_rs_unit(nc, P, 2 * (j - 1))]
                    fills += pu[4:8] + [_rs_unit(nc, P, 2 * (j - 1) + 1)]
                if j >= 2:
                    fills += [
                        _cast_unit(nc, P, 2 * (j - 2)),
                        _cast_unit(nc, P, 2 * (j - 2) + 1),
                    ]
                for u in _zip(a_units, fills):
                    u()

            # tail: last q-tile's projection, final two chunks, leftover casts
            pu = _proj_units(nc, P, NQT - 1)
            for u in pu[0:4]:
                u()
            _rs_unit(nc, P, 2 * (NQT - 1))()
            for u in pu[4:8]:
                u()
            _rs_unit(nc, P, 2 * (NQT - 1) + 1)()
            for k in (4, 5, 6, 7):
                _cast_unit(nc, P, k)()

    nc.compile()
    return nc


def _host_consts():
    ki = np.arange(128)[:, None]
    qj = np.arange(128)[None, :]
    masks = (qj >= ki).astype(ml_dtypes.bfloat16)  # [128, 128] diagonal band
    return masks


def _in_maps(x, w_qkv, w_proj):
    masks = _host_consts()
    xT = {}
    wq16 = {}
    wp16 = {}
    maps = []
    for c in range(8):
        b, g = c // 2, c % 2
        if b not in xT:
            xT[b] = np.ascontiguousarray(x[b].T).astype(ml_dtypes.bfloat16)
        if g not in wq16:
            wq = w_qkv[:, g * GD : (g + 1) * GD]
            wk = w_qkv[:, D + g * GD : D + (g + 1) * GD]
            wv = w_qkv[:, 2 * D + g * GD : 2 * D + (g + 1) * GD]
            wq16[g] = np.ascontiguousarray(
                np.concatenate([wq, wk, wv], axis=1)
            ).astype(ml_dtypes.bfloat16)
            wp16[g] = np.ascontiguousarray(
                w_proj[g * GD : (g + 1) * GD, :]
            ).astype(ml_dtypes.bfloat16)
        maps.append(
            {"xT": xT[b], "wqkv": wq16[g], "wproj": wp16[g], "masks": masks}
        )
    return maps


def kernel(x, w_qkv, w_proj):
    x = np.ascontiguousarray(x, dtype=np.float32)
    w_qkv = np.ascontiguousarray(w_qkv, dtype=np.float32)
    w_proj = np.ascontiguousarray(w_proj, dtype=np.float32)
    if "nc" not in _NC_CACHE:
        _NC_CACHE["nc"] = _build_nc()
    nc = _NC_CACHE["nc"]
    r = run_bass_kernel_spmd(nc, _in_maps(x, w_qkv, w_proj), list(range(8)))
    out = np.empty((4, S, D), np.float32)
    for b in range(4):
        for rk in range(2):
            o = r.results[2 * b + rk]["out"]
            for k in range(NCH):
                out[b, k * 256 + rk * 128 : k * 256 + rk * 128 + 128] = o[
                    k * 128 : (k + 1) * 128
                ]
    return out


# revision 7
# speedup vs baseline: 1.2244x; 1.0333x over previous
"""Causal self-attention kernel for Trainium2, 8 NeuronCores.

Sharding: DP4 x TP2. Core c = 2*b + g handles batch b (2048 tokens) and
head-group g (8 of 16 heads). Per core:
  - x arrives pre-transposed AND pre-cast to bf16 on the host (d_model on
    partitions), so startup is plain parallel DMA (no xbar transposes);
    the first q-tile's x slices and the Q columns of w_qkv are DMA'd first
    so the PE can start within a few microseconds,
  - QKV matmuls in bf16: Q,K dim-major ([head_dim, tokens]), V token-major
    65 wide (64 dims + a ones column -> softmax denominator for free),
  - attention per head pair: scores^T = K_h^T-tile @ Q_h in [k, q] layout
    with both heads' QK matmuls in different PE row groups (concurrent),
    one wide exp on ACT (1/sqrt(64) folded into its scale) into bf16 probs,
    causal handling by skipping fully-masked tiles, sub-tile column ranges
    on the diagonal band (scores, exp AND att@V all restricted to off:512),
    a 0/1 mask multiply on the 128-wide diagonal band,
  - the attention inner loop is software-pipelined (scores for c-tile n+1
    are emitted before att@V for tile n) and each head pair's epilogue is
    held back until the NEXT head pair's first scores have been issued, so
    the in-order PE queue never blocks on ACT or on the epilogue chain,
  - the epilogue never touches ACT: DVE copies the denominator row
    (lane-aligned at partition 64), DVE fast-reciprocal, gpsimd partition
    broadcast, DVE scale into bf16 dim-major yT,
  - QKV for later token tiles and projection partials for earlier tiles
    are zipped between attention units to keep the PE warm while ACT
    drains the exps (the last, largest attention tile gets two projection
    tiles' worth of fill so the PE never idles into a HAM re-throttle),
  - bf16 projection partials are summed across the core pair with chunked
    ReduceScatter (pipelined behind attention; the final q-tile is one
    bigger chunk so only a single collective sits in the tail),
  - host assembles the 4x2048x1024 output from the 8 interleaved shards.

Everything (shapes, sharding) is hardcoded for
x: [4, 2048, 1024], w_qkv: [1024, 3072], w_proj: [1024, 1024], f32.
"""

import ml_dtypes
import numpy as np

import concourse.bacc as bacc
import concourse.mybir as mybir
import concourse.tile as tile
from concourse.bass_utils import run_bass_kernel_spmd

F32 = mybir.dt.float32
BF16 = mybir.dt.bfloat16

S = 2048  # tokens per core (one batch element)
D = 1024  # d_model
HL = 8  # heads per core (local)
HD = 64  # head dim
GD = HL * HD  # 512, head-group dim
VW = HD + 1  # V row width: 64 dims + ones column (denominator)
NQT = S // 512  # 4 q-tiles of 512
NDM = D // 128  # 8 d_model chunks
NTOK = S // 128  # 16 token tiles of 128
RG = [[0, 1], [2, 3], [4, 5], [6, 7]]
# ReduceScatter chunks (start_row, n_rows): 6x256 for q-tiles 0-2, one 512
# for q-tile 3 (single collective in the tail)
CHUNKS = [(k * 256, 256) for k in range(6)] + [(1536, 512)]

_NC_CACHE = {}


def _qkv_units(nc, P, n):
    """QKV matmul chains for token tile n, as separately emittable units."""
    units = []

    def qk_chain(m):
        def emit():
            ps = P.b1_ps.tile([128, 512], F32, tag="b1", name="qkps")
            for k in range(NDM):
                nc.tensor.matmul(
                    ps,
                    P.w_sb[:, k, m * 128 : (m + 1) * 128],
                    P.xT[:, k, n * 512 : (n + 1) * 512],
                    start=(k == 0),
                    stop=(k == NDM - 1),
                )
            nc.vector.tensor_copy(
                out=P.qkT[:, m, n * 512 : (n + 1) * 512], in_=ps
            )

        return emit

    def v_chain(t4):
        def emit():
            t = n * 4 + t4
            ps = P.b1_ps.tile([128, 512], F32, tag="b1", name="vps")
            for k in range(NDM):
                nc.tensor.matmul(
                    ps,
                    P.xT[:, k, t * 128 : (t + 1) * 128],
                    P.w_sb[:, k, 2 * GD : 3 * GD],
                    start=(k == 0),
                    stop=(k == NDM - 1),
                )
            nc.vector.tensor_copy(
                out=P.v_sb[:, t, :, 0:HD],
                in_=ps.rearrange("p (h d) -> p h d", h=HL),
            )

        return emit

    for m in range(2 * GD // 128):
        units.append(qk_chain(m))
    for t4 in range(4):
        units.append(v_chain(t4))
    return units


def _attn_units(nc, P, j, pending):
    """Attention units for q-tile j, software-pipelined per head pair.
    Each head pair's epilogue is deferred into the next head pair's head
    (after its first two score units) via the `pending` 1-slot box."""
    units = []
    ncol = 4 * j + 4
    for hp in range(HL // 2):
        state = {}

        def alloc(state=state):
            state["yps"] = P.y_ps.tile(
                [128, 2, 512], F32, tag="yps", name="yps", bufs=1
            )

        def sc(c, hp=hp, state=state):
            def emit():
                d = c - 4 * j  # >= 0 on the diagonal band
                off = max(d, 0) * 128  # columns below off are fully masked
                sps2 = P.attn_ps.tile(
                    [128, 2, 512], F32, tag="sps2", name="sps2"
                )
                for hi in range(2):
                    h = 2 * hp + hi
                    po = (h % 2) * 64
                    nc.tensor.matmul(
                        sps2[:, hi, off:512],
                        P.qkT[po : po + 64, 4 + h // 2, c * 128 : (c + 1) * 128],
                        P.qkT[po : po + 64, h // 2, j * 512 + off : (j + 1) * 512],
                        start=True,
                        stop=True,
                    )
                probs2 = P.probs_p.tile(
                    [128, 2, 512], BF16, tag="probs", name="probs"
                )
                nc.scalar.activation(
                    out=probs2[:, :, off:512],
                    in_=sps2[:, :, off:512],
                    func=mybir.ActivationFunctionType.Exp,
                    scale=0.125,
                )
                if d >= 0:
                    for hi in range(2):
                        nc.vector.tensor_mul(
                            probs2[:, hi, off : off + 128],
                            probs2[:, hi, off : off + 128],
                            P.mask_sb,
                        )
                state[c] = (probs2, off)

            return emit

        def av(c, hp=hp, state=state):
            def emit():
                probs2, off = state.pop(c)
                yps = state["yps"]
                for hi in range(2):
                    h = 2 * hp + hi
                    nc.tensor.matmul(
                        yps[0:VW, hi, off:512],
                        P.v_sb[:, c, h, :],
                        probs2[:, hi, off:512],
                        start=(c == 0),
                        stop=(c == ncol - 1),
                    )

            return emit

        def epilogue(hp=hp, state=state, j=j):
            # ones-row out of PSUM to partition 0 (ACT can shift
            # partitions), DVE fast reciprocal, gpsimd partition broadcast,
            # DVE scale into bf16 dim-major yT
            yps = state["yps"]
            dsb = P.den_p.tile([1, 2, 512], F32, tag="den", name="den")
            nc.scalar.activation(
                out=dsb,
                in_=yps[HD : HD + 1, :, :],
                func=mybir.ActivationFunctionType.Copy,
            )
            nc.vector.reciprocal_approx_fast(out=dsb, in_=dsb)
            denb = P.den_p.tile([HD, 2, 512], F32, tag="denb", name="denb")
            nc.gpsimd.partition_broadcast(denb, dsb)
            for hi in range(2):
                h = 2 * hp + hi
                po = (h % 2) * 64
                nc.vector.tensor_mul(
                    P.yT[po : po + 64, h // 2, j * 512 : (j + 1) * 512],
                    yps[0:HD, hi, :],
                    denb[:, hi, :],
                )

        units.append(alloc)
        units.append(sc(0))
        units.append(sc(1))
        if pending[0] is not None:
            units.append(pending[0])
            pending[0] = None
        units.append(av(0))
        for c in range(2, ncol):
            units.append(sc(c))
            units.append(av(c - 1))
        units.append(av(ncol - 1))
        pending[0] = epilogue
    return units


def _proj_units(nc, P, j):
    """bf16 projection partials for q-tile j -> cc_in rows, 2 units/tile."""
    units = []
    for mt in range(4 * j, 4 * j + 4):
        state = {}

        def half(nh, mt=mt, state=state):
            def emit():
                if nh == 0:
                    state["osb"] = P.out_p.tile(
                        [128, D], BF16, tag="osb", name="osb"
                    )
                osb = state["osb"]
                ps = P.b1_ps.tile([128, 512], F32, tag="b1", name="ops")
                for kk in range(GD // 128):
                    nc.tensor.matmul(
                        ps,
                        P.yT[:, kk, mt * 128 : (mt + 1) * 128],
                        P.wp_sb[:, kk, nh * 512 : (nh + 1) * 512],
                        start=(kk == 0),
                        stop=(kk == GD // 128 - 1),
                    )
                nc.vector.tensor_copy(
                    out=osb[:, nh * 512 : (nh + 1) * 512], in_=ps
                )
                if nh == 1:
                    nc.sync.dma_start(
                        out=P.cc_in[mt * 128 : (mt + 1) * 128, :], in_=osb
                    )

            return emit

        units.append(half(0))
        units.append(half(1))
    return units


def _rs_unit(nc, P, k):
    """ReduceScatter of chunk k; my half-size shard -> cc_red."""

    def emit():
        lo, n = CHUNKS[k]
        nc.gpsimd.collective_compute(
            "ReduceScatter",
            mybir.AluOpType.add,
            replica_groups=RG,
            ins=[P.cc_in[lo : lo + n, :].opt()],
            outs=[P.cc_red[lo // 2 : lo // 2 + n // 2, :].opt()],
        )

    return emit


def _cast_units(nc, P, k):
    """Upcast my bf16 shard of chunk k to fp32 and DMA it out."""
    lo, n = CHUNKS[k]
    units = []
    for t in range(n // 256):
        def emit(t=t):
            row = lo // 2 + t * 128
            ci = P.cast_p.tile([128, D], BF16, tag="ci", name="ci")
            nc.sync.dma_start(out=ci, in_=P.cc_red[row : row + 128, :])
            co = P.cast_p.tile([128, D], F32, tag="co", name="co")
            nc.vector.tensor_copy(out=co, in_=ci)
            nc.sync.dma_start(out=P.out[row : row + 128, :], in_=co)

        units.append(emit)
    return units


def _zip(a_units, fill_units):
    """Spread fill_units evenly between a_units (fills go to the PE's idle
    slots while ACT drains the exps)."""
    na, nf = len(a_units), len(fill_units)
    if nf == 0:
        return list(a_units)
    pos = [int(na * (f + 1) / (nf + 1)) for f in range(nf)]
    out = []
    fi = 0
    for i, u in enumerate(a_units):
        out.append(u)
        while fi < nf and pos[fi] <= i:
            out.append(fill_units[fi])
            fi += 1
    out.extend(fill_units[fi:])
    return out


class _Ctx:
    pass


def _build_nc():
    nc = bacc.Bacc(None, num_devices=8)
    P = _Ctx()

    xTd = nc.dram_tensor("xT", [D, S], BF16, kind="ExternalInput").ap()
    wqkv = nc.dram_tensor("wqkv", [D, 3 * GD], BF16, kind="ExternalInput").ap()
    wproj = nc.dram_tensor("wproj", [GD, D], BF16, kind="ExternalInput").ap()
    masks = nc.dram_tensor("masks", [128, 128], BF16, kind="ExternalInput").ap()
    P.out = nc.dram_tensor("out", [S // 2, D], F32, kind="ExternalOutput").ap()

    with tile.TileContext(nc) as tc:
        with (
            tc.tile_pool(name="const", bufs=1) as const,
            tc.tile_pool(name="w_p", bufs=1) as w_p,
            tc.tile_pool(name="big_p", bufs=1) as big_p,
            tc.tile_pool(name="probs_p", bufs=8) as probs_p,
            tc.tile_pool(name="den_p", bufs=2) as den_p,
            tc.tile_pool(name="out_p", bufs=2) as out_p,
            tc.tile_pool(name="cast_p", bufs=2) as cast_p,
            tc.tile_pool(name="b1_ps", bufs=2, space="PSUM") as b1_ps,
            tc.tile_pool(name="attn_ps", bufs=2, space="PSUM") as attn_ps,
            tc.tile_pool(name="y_ps", bufs=1, space="PSUM") as y_ps,
            tc.tile_pool(name="dram", bufs=1, space="DRAM") as dram,
        ):
            P.probs_p, P.den_p, P.out_p, P.cast_p = probs_p, den_p, out_p, cast_p
            P.b1_ps, P.attn_ps, P.y_ps = b1_ps, attn_ps, y_ps

            # Startup: plain DMAs only. First the x slices and w columns the
            # first QKV chains need, then everything else.
            P.xT = big_p.tile([128, NDM, S], BF16, name="xT")
            P.w_sb = w_p.tile([128, NDM, 3 * GD], BF16, name="w_sb")
            for k in range(NDM):
                nc.sync.dma_start(
                    out=P.xT[:, k, 0:512], in_=xTd[k * 128 : (k + 1) * 128, 0:512]
                )
            for k in range(NDM):  # Q columns
                nc.sync.dma_start(
                    out=P.w_sb[:, k, 0:GD],
                    in_=wqkv[k * 128 : (k + 1) * 128, 0:GD],
                )
            for k in range(NDM):  # K and V columns
                nc.sync.dma_start(
                    out=P.w_sb[:, k, GD : 3 * GD],
                    in_=wqkv[k * 128 : (k + 1) * 128, GD : 3 * GD],
                )
            P.mask_sb = const.tile([128, 128], BF16, name="mask_sb")
            nc.sync.dma_start(out=P.mask_sb, in_=masks)
            # preload the exp table while DMAs run
            aw = const.tile([1, 2], F32, name="actwarm")
            nc.vector.memset(aw, 0.0)
            nc.scalar.activation(
                out=aw, in_=aw, func=mybir.ActivationFunctionType.Exp
            )
            P.wp_sb = w_p.tile([128, GD // 128, D], BF16, name="wp_sb")
            for kk in range(GD // 128):
                nc.sync.dma_start(
                    out=P.wp_sb[:, kk, :], in_=wproj[kk * 128 : (kk + 1) * 128, :]
                )
            for k in range(NDM):
                nc.sync.dma_start(
                    out=P.xT[:, k, 512:S], in_=xTd[k * 128 : (k + 1) * 128, 512:S]
                )

            P.qkT = big_p.tile([128, 2 * GD // 128, S], BF16, name="qkT")
            P.v_sb = big_p.tile([128, NTOK, HL, VW], BF16, name="v_sb")
            nc.vector.memset(P.v_sb[:, :, :, HD : HD + 1], 1.0)
            P.yT = big_p.tile([128, GD // 128, S], BF16, name="yT")

            P.cc_in = dram.tile([S, D], BF16, name="cc_in")
            P.cc_red = dram.tile([S // 2, D], BF16, name="cc_red")

            for u in _qkv_units(nc, P, 0):
                u()

            pending = [None]
            fills_by_tile = {
                0: lambda: _qkv_units(nc, P, 1),
                1: lambda: _qkv_units(nc, P, 2),
                2: lambda: (
                    _qkv_units(nc, P, 3)
                    + _proj_units(nc, P, 0)[0:4]
                    + [_rs_unit(nc, P, 0)]
                    + _proj_units(nc, P, 0)[4:8]
                    + [_rs_unit(nc, P, 1)]
                ),
                3: lambda: (
                    _proj_units(nc, P, 1)[0:4]
                    + [_rs_unit(nc, P, 2)]
                    + _proj_units(nc, P, 1)[4:8]
                    + [_rs_unit(nc, P, 3)]
                    + _cast_units(nc, P, 0)
                    + _cast_units(nc, P, 1)
                    + _proj_units(nc, P, 2)[0:4]
                    + [_rs_unit(nc, P, 4)]
                    + _proj_units(nc, P, 2)[4:8]
                    + [_rs_unit(nc, P, 5)]
                    + _cast_units(nc, P, 2)
                    + _cast_units(nc, P, 3)
                ),
            }
            for j in range(NQT):
                a_units = _attn_units(nc, P, j, pending)
                for u in _zip(a_units, fills_by_tile[j]()):
                    u()

            # tail: final epilogue, last q-tile's projection, one big
            # ReduceScatter, remaining casts
            pending[0]()
            pending[0] = None
            pu = _proj_units(nc, P, 3)
            for i, u in enumerate(pu):
                u()
                if i == 3:
                    for cu in _cast_units(nc, P, 4):
                        cu()
            _rs_unit(nc, P, 6)()
            for cu in _cast_units(nc, P, 5) + _cast_units(nc, P, 6):
                cu()

    nc.compile()
    return nc


def _host_consts():
    ki = np.arange(128)[:, None]
    qj = np.arange(128)[None, :]
    masks = (qj >= ki).astype(ml_dtypes.bfloat16)  # [128, 128] diagonal band
    return masks


def _in_maps(x, w_qkv, w_proj):
    masks = _host_consts()
    xT = {}
    wq16 = {}
    wp16 = {}
    maps = []
    for c in range(8):
        b, g = c // 2, c % 2
        if b not in xT:
            xT[b] = np.ascontiguousarray(x[b].T).astype(ml_dtypes.bfloat16)
        if g not in wq16:
            wq = w_qkv[:, g * GD : (g + 1) * GD]
            wk = w_qkv[:, D + g * GD : D + (g + 1) * GD]
            wv = w_qkv[:, 2 * D + g * GD : 2 * D + (g + 1) * GD]
            wq16[g] = np.ascontiguousarray(
                np.concatenate([wq, wk, wv], axis=1)
            ).astype(ml_dtypes.bfloat16)
            wp16[g] = np.ascontiguousarray(
                w_proj[g * GD : (g + 1) * GD, :]
            ).astype(ml_dtypes.bfloat16)
        maps.append(
            {"xT": xT[b], "wqkv": wq16[g], "wproj": wp16[g], "masks": masks}
        )
    return maps


def kernel(x, w_qkv, w_proj):
    x = np.ascontiguousarray(x, dtype=np.float32)
    w_qkv = np.ascontiguousarray(w_qkv, dtype=np.float32)
    w_proj = np.ascontiguousarray(w_proj, dtype=np.float32)
    if "nc" not in _NC_CACHE:
        _NC_CACHE["nc"] = _build_nc()
    nc = _NC_CACHE["nc"]
    r = run_bass_kernel_spmd(nc, _in_maps(x, w_qkv, w_proj), list(range(8)))
    out = np.empty((4, S, D), np.float32)
    for b in range(4):
        for rk in range(2):
            o = r.results[2 * b + rk]["out"]
            for k, (lo, n) in enumerate(CHUNKS):
                h = n // 2
                out[b, lo + rk * h : lo + (rk + 1) * h] = o[
                    lo // 2 : lo // 2 + h
                ]
    return out


# revision 9
# speedup vs baseline: 1.2273x; 1.0024x over previous
"""Causal self-attention kernel for Trainium2, 8 NeuronCores.

Sharding: DP4 x TP2. Core c = 2*b + g handles batch b (2048 tokens) and
head-group g (8 of 16 heads). Per core:
  - x arrives pre-transposed AND pre-cast to bf16 on the host (d_model on
    partitions), so startup is plain parallel DMA (no xbar transposes);
    the first q-tile's x slices and the Q columns of w_qkv are DMA'd first
    so the PE can start within a few microseconds,
  - QKV matmuls in bf16: Q,K dim-major ([head_dim, tokens]), V token-major
    65 wide (64 dims + a ones column -> softmax denominator for free),
  - attention per head pair: scores^T = K_h^T-tile @ Q_h in [k, q] layout
    with both heads' QK matmuls in different PE row groups (concurrent),
    one wide exp on ACT (1/sqrt(64) folded into its scale) into bf16 probs,
    causal handling by skipping fully-masked tiles, sub-tile column ranges
    on the diagonal band (scores, exp AND att@V all restricted to off:512),
    a 0/1 mask multiply on the 128-wide diagonal band,
  - the attention inner loop is software-pipelined (scores for c-tile n+1
    are emitted before att@V for tile n) and each head pair's epilogue is
    held back until the NEXT head pair's first scores have been issued, so
    the in-order PE queue never blocks on ACT or on the epilogue chain,
  - the epilogue never touches ACT: DVE copies the denominator row
    (lane-aligned at partition 64), DVE fast-reciprocal, gpsimd partition
    broadcast, DVE scale into bf16 dim-major yT,
  - QKV for later token tiles and projection partials for earlier tiles
    are zipped between attention units to keep the PE warm while ACT
    drains the exps (the last, largest attention tile gets two projection
    tiles' worth of fill so the PE never idles into a HAM re-throttle),
  - bf16 projection partials are summed across the core pair with chunked
    ReduceScatter (pipelined behind attention; the final q-tile is one
    bigger chunk so only a single collective sits in the tail),
  - host assembles the 4x2048x1024 output from the 8 interleaved shards.

Everything (shapes, sharding) is hardcoded for
x: [4, 2048, 1024], w_qkv: [1024, 3072], w_proj: [1024, 1024], f32.
"""

import ml_dtypes
import numpy as np

import concourse.bacc as bacc
import concourse.mybir as mybir
import concourse.tile as tile
from concourse.bass_utils import run_bass_kernel_spmd

F32 = mybir.dt.float32
BF16 = mybir.dt.bfloat16

S = 2048  # tokens per core (one batch element)
D = 1024  # d_model
HL = 8  # heads per core (local)
HD = 64  # head dim
GD = HL * HD  # 512, head-group dim
VW = HD + 1  # V row width: 64 dims + ones column (denominator)
NQT = S // 512  # 4 q-tiles of 512
NDM = D // 128  # 8 d_model chunks
NTOK = S // 128  # 16 token tiles of 128
RG = [[0, 1], [2, 3], [4, 5], [6, 7]]
# ReduceScatter chunks (start_row, n_rows): 6x256 for q-tiles 0-2, one 512
# for q-tile 3 (single collective in the tail)
CHUNKS = [(k * 256, 256) for k in range(6)] + [(1536, 512)]

_NC_CACHE = {}


def _qkv_units(nc, P, n):
    """QKV matmul chains for token tile n, as separately emittable units."""
    units = []

    def qk_chain(m):
        def emit():
            ps = P.b1_ps.tile([128, 512], F32, tag="b1", name="qkps")
            for k in range(NDM):
                nc.tensor.matmul(
                    ps,
                    P.w_sb[:, k, m * 128 : (m + 1) * 128],
                    P.xT[:, k, n * 512 : (n + 1) * 512],
                    start=(k == 0),
                    stop=(k == NDM - 1),
                )
            nc.vector.tensor_copy(
                out=P.qkT[:, m, n * 512 : (n + 1) * 512], in_=ps
            )

        return emit

    def v_chain(t4):
        def emit():
            t = n * 4 + t4
            ps = P.b1_ps.tile([128, 512], F32, tag="b1", name="vps")
            for k in range(NDM):
                nc.tensor.matmul(
                    ps,
                    P.xT[:, k, t * 128 : (t + 1) * 128],
                    P.w_sb[:, k, 2 * GD : 3 * GD],
                    start=(k == 0),
                    stop=(k == NDM - 1),
                )
            nc.vector.tensor_copy(
                out=P.v_sb[:, t, :, 0:HD],
                in_=ps.rearrange("p (h d) -> p h d", h=HL),
            )

        return emit

    for m in range(2 * GD // 128):
        units.append(qk_chain(m))
    for t4 in range(4):
        units.append(v_chain(t4))
    return units


def _attn_units(nc, P, j, pending):
    """Attention units for q-tile j, software-pipelined per head pair.
    Each head pair's epilogue is deferred into the next head pair's head
    (after its first two score units) via the `pending` 1-slot box."""
    units = []
    ncol = 4 * j + 4
    for hp in range(HL // 2):
        state = {}

        def alloc(state=state):
            state["yps"] = P.y_ps.tile(
                [128, 2, 512], F32, tag="yps", name="yps", bufs=1
            )

        def sc(c, hp=hp, state=state):
            def emit():
                d = c - 4 * j  # >= 0 on the diagonal band
                off = max(d, 0) * 128  # columns below off are fully masked
                sps2 = P.attn_ps.tile(
                    [128, 2, 512], F32, tag="sps2", name="sps2"
                )
                for hi in range(2):
                    h = 2 * hp + hi
                    po = (h % 2) * 64
                    nc.tensor.matmul(
                        sps2[:, hi, off:512],
                        P.qkT[po : po + 64, 4 + h // 2, c * 128 : (c + 1) * 128],
                        P.qkT[po : po + 64, h // 2, j * 512 + off : (j + 1) * 512],
                        start=True,
                        stop=True,
                    )
                probs2 = P.probs_p.tile(
                    [128, 2, 512], BF16, tag="probs", name="probs"
                )
                nc.scalar.activation(
                    out=probs2[:, :, off:512],
                    in_=sps2[:, :, off:512],
                    func=mybir.ActivationFunctionType.Exp,
                    scale=0.125,
                )
                if d >= 0:
                    for hi in range(2):
                        nc.vector.tensor_mul(
                            probs2[:, hi, off : off + 128],
                            probs2[:, hi, off : off + 128],
                            P.mask_sb,
                        )
                state[c] = (probs2, off)

            return emit

        def av(c, hp=hp, state=state):
            def emit():
                probs2, off = state.pop(c)
                yps = state["yps"]
                for hi in range(2):
                    h = 2 * hp + hi
                    nc.tensor.matmul(
                        yps[0:VW, hi, off:512],
                        P.v_sb[:, c, h, :],
                        probs2[:, hi, off:512],
                        start=(c == 0),
                        stop=(c == ncol - 1),
                    )

            return emit

        def epilogue(hp=hp, state=state, j=j):
            # ones-row out of PSUM to partition 0 (ACT can shift
            # partitions), DVE fast reciprocal, gpsimd partition broadcast,
            # DVE scale into bf16 dim-major yT
            yps = state["yps"]
            dsb = P.den_p.tile([1, 2, 512], F32, tag="den", name="den")
            nc.scalar.activation(
                out=dsb,
                in_=yps[HD : HD + 1, :, :],
                func=mybir.ActivationFunctionType.Copy,
            )
            nc.vector.reciprocal_approx_fast(out=dsb, in_=dsb)
            denb = P.den_p.tile([HD, 2, 512], F32, tag="denb", name="denb")
            nc.gpsimd.partition_broadcast(denb, dsb)
            for hi in range(2):
                h = 2 * hp + hi
                po = (h % 2) * 64
                nc.vector.tensor_mul(
                    P.yT[po : po + 64, h // 2, j * 512 : (j + 1) * 512],
                    yps[0:HD, hi, :],
                    denb[:, hi, :],
                )

        units.append(alloc)
        units.append(sc(0))
        units.append(sc(1))
        if pending[0] is not None:
            units.append(pending[0])
            pending[0] = None
        units.append(av(0))
        for c in range(2, ncol):
            units.append(sc(c))
            units.append(av(c - 1))
        units.append(av(ncol - 1))
        pending[0] = epilogue
    return units


def _proj_units(nc, P, j):
    """bf16 projection partials for q-tile j -> cc_in rows, 2 units/tile."""
    units = []
    for mt in range(4 * j, 4 * j + 4):
        state = {}

        def half(nh, mt=mt, state=state):
            def emit():
                if nh == 0:
                    state["osb"] = P.out_p.tile(
                        [128, D], BF16, tag="osb", name="osb"
                    )
                osb = state["osb"]
                ps = P.b1_ps.tile([128, 512], F32, tag="b1", name="ops")
                for kk in range(GD // 128):
                    nc.tensor.matmul(
                        ps,
                        P.yT[:, kk, mt * 128 : (mt + 1) * 128],
                        P.wp_sb[:, kk, nh * 512 : (nh + 1) * 512],
                        start=(kk == 0),
                        stop=(kk == GD // 128 - 1),
                    )
                nc.vector.tensor_copy(
                    out=osb[:, nh * 512 : (nh + 1) * 512], in_=ps
                )
                if nh == 1:
                    nc.sync.dma_start(
                        out=P.cc_in[mt * 128 : (mt + 1) * 128, :], in_=osb
                    )

            return emit

        units.append(half(0))
        units.append(half(1))
    return units


def _rs_unit(nc, P, k):
    """ReduceScatter of chunk k; my half-size shard -> cc_red."""

    def emit():
        lo, n = CHUNKS[k]
        nc.gpsimd.collective_compute(
            "ReduceScatter",
            mybir.AluOpType.add,
            replica_groups=RG,
            ins=[P.cc_in[lo : lo + n, :].opt()],
            outs=[P.cc_red[lo // 2 : lo // 2 + n // 2, :].opt()],
        )

    return emit


def _cast_units(nc, P, k):
    """Upcast my bf16 shard of chunk k to fp32 and DMA it out."""
    lo, n = CHUNKS[k]
    units = []
    for t in range(n // 256):
        def emit(t=t):
            row = lo // 2 + t * 128
            ci = P.cast_p.tile([128, D], BF16, tag="ci", name="ci")
            nc.sync.dma_start(out=ci, in_=P.cc_red[row : row + 128, :])
            co = P.cast_p.tile([128, D], F32, tag="co", name="co")
            nc.vector.tensor_copy(out=co, in_=ci)
            nc.sync.dma_start(out=P.out[row : row + 128, :], in_=co)

        units.append(emit)
    return units


def _zip(a_units, fill_units):
    """Spread fill_units evenly between a_units (fills go to the PE's idle
    slots while ACT drains the exps)."""
    na, nf = len(a_units), len(fill_units)
    if nf == 0:
        return list(a_units)
    pos = [int(na * (f + 1) / (nf + 1)) for f in range(nf)]
    out = []
    fi = 0
    for i, u in enumerate(a_units):
        out.append(u)
        while fi < nf and pos[fi] <= i:
            out.append(fill_units[fi])
            fi += 1
    out.extend(fill_units[fi:])
    return out


class _Ctx:
    pass


def _build_nc():
    nc = bacc.Bacc(None, num_devices=8)
    P = _Ctx()

    xTd = nc.dram_tensor("xT", [D, S], BF16, kind="ExternalInput").ap()
    wqkv = nc.dram_tensor("wqkv", [D, 3 * GD], BF16, kind="ExternalInput").ap()
    wproj = nc.dram_tensor("wproj", [GD, D], BF16, kind="ExternalInput").ap()
    masks = nc.dram_tensor("masks", [128, 128], BF16, kind="ExternalInput").ap()
    P.out = nc.dram_tensor("out", [S // 2, D], F32, kind="ExternalOutput").ap()

    with tile.TileContext(nc) as tc:
        with (
            tc.tile_pool(name="const", bufs=1) as const,
            tc.tile_pool(name="w_p", bufs=1) as w_p,
            tc.tile_pool(name="big_p", bufs=1) as big_p,
            tc.tile_pool(name="probs_p", bufs=8) as probs_p,
            tc.tile_pool(name="den_p", bufs=2) as den_p,
            tc.tile_pool(name="out_p", bufs=2) as out_p,
            tc.tile_pool(name="cast_p", bufs=2) as cast_p,
            tc.tile_pool(name="b1_ps", bufs=2, space="PSUM") as b1_ps,
            tc.tile_pool(name="attn_ps", bufs=2, space="PSUM") as attn_ps,
            tc.tile_pool(name="y_ps", bufs=1, space="PSUM") as y_ps,
            tc.tile_pool(name="dram", bufs=1, space="DRAM") as dram,
        ):
            P.probs_p, P.den_p, P.out_p, P.cast_p = probs_p, den_p, out_p, cast_p
            P.b1_ps, P.attn_ps, P.y_ps = b1_ps, attn_ps, y_ps

            # Startup: plain DMAs only, k-interleaved so QKV chain m=0 can
            # start its k-th accumulation step as soon as slice k lands.
            P.xT = big_p.tile([128, NDM, S], BF16, name="xT")
            P.w_sb = w_p.tile([128, NDM, 3 * GD], BF16, name="w_sb")
            P.mask_sb = const.tile([128, 128], BF16, name="mask_sb")
            nc.sync.dma_start(out=P.mask_sb, in_=masks)
            for k in range(NDM):
                nc.sync.dma_start(
                    out=P.xT[:, k, 0:512], in_=xTd[k * 128 : (k + 1) * 128, 0:512]
                )
                nc.sync.dma_start(
                    out=P.w_sb[:, k, 0:GD],
                    in_=wqkv[k * 128 : (k + 1) * 128, 0:GD],
                )
            # preload the exp table while DMAs run
            aw = const.tile([1, 2], F32, name="actwarm")
            nc.vector.memset(aw, 0.0)
            nc.scalar.activation(
                out=aw, in_=aw, func=mybir.ActivationFunctionType.Exp
            )
            for k in range(NDM):  # K columns
                nc.sync.dma_start(
                    out=P.w_sb[:, k, GD : 2 * GD],
                    in_=wqkv[k * 128 : (k + 1) * 128, GD : 2 * GD],
                )
            for k in range(NDM):  # V columns
                nc.sync.dma_start(
                    out=P.w_sb[:, k, 2 * GD : 3 * GD],
                    in_=wqkv[k * 128 : (k + 1) * 128, 2 * GD : 3 * GD],
                )
            P.wp_sb = w_p.tile([128, GD // 128, D], BF16, name="wp_sb")
            for kk in range(GD // 128):
                nc.sync.dma_start(
                    out=P.wp_sb[:, kk, :], in_=wproj[kk * 128 : (kk + 1) * 128, :]
                )
            for k in range(NDM):
                nc.sync.dma_start(
                    out=P.xT[:, k, 512:S], in_=xTd[k * 128 : (k + 1) * 128, 512:S]
                )

            P.qkT = big_p.tile([128, 2 * GD // 128, S], BF16, name="qkT")
            P.v_sb = big_p.tile([128, NTOK, HL, VW], BF16, name="v_sb")
            nc.vector.memset(P.v_sb[:, :, :, HD : HD + 1], 1.0)
            P.yT = big_p.tile([128, GD // 128, S], BF16, name="yT")

            P.cc_in = dram.tile([S, D], BF16, name="cc_in")
            P.cc_red = dram.tile([S // 2, D], BF16, name="cc_red")

            # QKV tile 0: only what attention (j0, hp0) needs up front; the
            # remaining chains become fill for the j0 attention region.
            q0 = _qkv_units(nc, P, 0)
            q0_lead = [q0[0], q0[4], q0[8], q0[9], q0[10], q0[11]]
            q0_rest = [q0[1], q0[5], q0[2], q0[6], q0[3], q0[7]]
            for u in q0_lead:
                u()

            pending = [None]
            fills_by_tile = {
                0: lambda: q0_rest + _qkv_units(nc, P, 1),
                1: lambda: _qkv_units(nc, P, 2),
                2: lambda: (
                    _qkv_units(nc, P, 3)
                    + _proj_units(nc, P, 0)[0:4]
                    + [_rs_unit(nc, P, 0)]
                    + _proj_units(nc, P, 0)[4:8]
                    + [_rs_unit(nc, P, 1)]
                ),
                3: lambda: (
                    _proj_units(nc, P, 1)[0:4]
                    + [_rs_unit(nc, P, 2)]
                    + _proj_units(nc, P, 1)[4:8]
                    + [_rs_unit(nc, P, 3)]
                    + _proj_units(nc, P, 2)[0:4]
                    + [_rs_unit(nc, P, 4)]
                    + _proj_units(nc, P, 2)[4:8]
                    + [_rs_unit(nc, P, 5)]
                ),
            }
            for j in range(NQT):
                a_units = _attn_units(nc, P, j, pending)
                for u in _zip(a_units, fills_by_tile[j]()):
                    u()

            # tail: final epilogue, last q-tile's projection, earlier casts
            # (their inputs are long since reduced; DVE is free here and the
            # out-DMAs overlap the final ReduceScatter), big RS, last cast
            pending[0]()
            pending[0] = None
            for u in _proj_units(nc, P, 3):
                u()
            for k in range(6):
                for cu in _cast_units(nc, P, k):
                    cu()
            _rs_unit(nc, P, 6)()
            for cu in _cast_units(nc, P, 6):
                cu()

    nc.compile()
    return nc


def _host_consts():
    ki = np.arange(128)[:, None]
    qj = np.arange(128)[None, :]
    masks = (qj >= ki).astype(ml_dtypes.bfloat16)  # [128, 128] diagonal band
    return masks


def _in_maps(x, w_qkv, w_proj):
    masks = _host_consts()
    xT = {}
    wq16 = {}
    wp16 = {}
    maps = []
    for c in range(8):
        b, g = c // 2, c % 2
        if b not in xT:
            xT[b] = np.ascontiguousarray(x[b].T).astype(ml_dtypes.bfloat16)
        if g not in wq16:
            wq = w_qkv[:, g * GD : (g + 1) * GD]
            wk = w_qkv[:, D + g * GD : D + (g + 1) * GD]
            wv = w_qkv[:, 2 * D + g * GD : 2 * D + (g + 1) * GD]
            wq16[g] = np.ascontiguousarray(
                np.concatenate([wq, wk, wv], axis=1)
            ).astype(ml_dtypes.bfloat16)
            wp16[g] = np.ascontiguousarray(
                w_proj[g * GD : (g + 1) * GD, :]
            ).astype(ml_dtypes.bfloat16)
        maps.append(
            {"xT": xT[b], "wqkv": wq16[g], "wproj": wp16[g], "masks": masks}
        )
    return maps


def kernel(x, w_qkv, w_proj):
    x = np.ascontiguousarray(x, dtype=np.float32)
    w_qkv = np.ascontiguousarray(w_qkv, dtype=np.float32)
    w_proj = np.ascontiguousarray(w_proj, dtype=np.float32)
    if "nc" not in _NC_CACHE:
        _NC_CACHE["nc"] = _build_nc()
    nc = _NC_CACHE["nc"]
    r = run_bass_kernel_spmd(nc, _in_maps(x, w_qkv, w_proj), list(range(8)))
    out = np.empty((4, S, D), np.float32)
    for b in range(4):
        for rk in range(2):
            o = r.results[2 * b + rk]["out"]
            for k, (lo, n) in enumerate(CHUNKS):
                h = n // 2
                out[b, lo + rk * h : lo + (rk + 1) * h] = o[
                    lo // 2 : lo // 2 + h
                ]
    return out


# revision 14
# speedup vs baseline: 1.2763x; 1.0399x over previous
"""Causal self-attention kernel for Trainium2, 8 NeuronCores.

Sharding: DP4 x TP2. Core c = 2*b + g handles batch b (2048 tokens) and
head-group g (8 of 16 heads). Per core:
  - x arrives pre-transposed AND pre-cast to bf16 on the host (d_model on
    partitions), so startup is plain parallel DMA (no xbar transposes);
    the first q-tile's x slices and the Q columns of w_qkv are DMA'd first
    so the PE can start within a few microseconds,
  - QKV matmuls in bf16: Q,K dim-major ([head_dim, tokens]), V token-major
    65 wide (64 dims + a ones column -> softmax denominator for free),
  - attention per head pair: scores^T = K_h^T-tile @ Q_h in [k, q] layout
    with both heads' QK matmuls in different PE row groups (concurrent),
    one wide exp on ACT (1/sqrt(64) folded into its scale) into bf16 probs,
    causal handling by skipping fully-masked tiles, sub-tile column ranges
    on the diagonal band (scores, exp AND att@V all restricted to off:512),
    a 0/1 mask multiply on the 128-wide diagonal band,
  - the attention inner loop is software-pipelined (scores for c-tile n+1
    are emitted before att@V for tile n) and each head pair's epilogue is
    held back until the NEXT head pair's first scores have been issued, so
    the in-order PE queue never blocks on ACT or on the epilogue chain,
  - the epilogue never touches ACT: DVE copies the denominator row
    (lane-aligned at partition 64), DVE fast-reciprocal, gpsimd partition
    broadcast, DVE scale into bf16 dim-major yT,
  - QKV for later token tiles and projection partials for earlier tiles
    are zipped between attention units to keep the PE warm while ACT
    drains the exps (the last, largest attention tile gets two projection
    tiles' worth of fill so the PE never idles into a HAM re-throttle),
  - bf16 projection partials are summed across the core pair with chunked
    ReduceScatter (pipelined behind attention; the final q-tile is one
    bigger chunk so only a single collective sits in the tail),
  - host assembles the 4x2048x1024 output from the 8 interleaved shards.

Everything (shapes, sharding) is hardcoded for
x: [4, 2048, 1024], w_qkv: [1024, 3072], w_proj: [1024, 1024], f32.
"""

import ml_dtypes
import numpy as np

import concourse.bacc as bacc
import concourse.mybir as mybir
import concourse.tile as tile
from concourse.tile import add_dep_helper
from concourse.bass_utils import run_bass_kernel_spmd

F32 = mybir.dt.float32
BF16 = mybir.dt.bfloat16

S = 2048  # tokens per core (one batch element)
D = 1024  # d_model
HL = 8  # heads per core (local)
HD = 64  # head dim
GD = HL * HD  # 512, head-group dim
VW = HD + 1  # V row width: 64 dims + ones column (denominator)
NQT = S // 512  # 4 q-tiles of 512
NDM = D // 128  # 8 d_model chunks
NTOK = S // 128  # 16 token tiles of 128
RG = [[0, 1], [2, 3], [4, 5], [6, 7]]
# ReduceScatter chunks (start_row, n_rows): one 512-row chunk per q-tile —
# the ~12us fixed cost per collective dominates, so fewer+larger wins, and
# spacing triggers one attention tile apart keeps the CC queue drained so
# gpsimd doorbell writes never block the epilogue broadcasts behind them
CHUNKS = [(k * 512, 512) for k in range(4)]

_NC_CACHE = {}


def _qkv_units(nc, P, n):
    """QKV matmul chains for token tile n, as separately emittable units."""
    units = []

    def qk_chain(m):
        def emit():
            ps = P.b1_ps.tile([128, 512], F32, tag="b1", name="qkps")
            for k in range(NDM):
                nc.tensor.matmul(
                    ps,
                    P.w_sb[:, k, m * 128 : (m + 1) * 128],
                    P.xT[:, k, n * 512 : (n + 1) * 512],
                    start=(k == 0),
                    stop=(k == NDM - 1),
                )
            nc.vector.tensor_copy(
                out=P.qkT[:, m, n * 512 : (n + 1) * 512], in_=ps
            )

        return emit

    def v_chain(t4):
        def emit():
            t = n * 4 + t4
            ps = P.b1_ps.tile([128, 512], F32, tag="b1", name="vps")
            for k in range(NDM):
                nc.tensor.matmul(
                    ps,
                    P.xT[:, k, t * 128 : (t + 1) * 128],
                    P.w_sb[:, k, 2 * GD : 3 * GD],
                    start=(k == 0),
                    stop=(k == NDM - 1),
                )
            nc.vector.tensor_copy(
                out=P.v_sb[:, t, :, 0:HD],
                in_=ps.rearrange("p (h d) -> p h d", h=HL),
            )

        return emit

    for m in range(2 * GD // 128):
        units.append(qk_chain(m))
    for t4 in range(4):
        units.append(v_chain(t4))
    return units


def _attn_units(nc, P, j, pending):
    """Attention units for q-tile j, software-pipelined per head pair.
    Each head pair's epilogue is deferred into the next head pair's head
    (after its first two score units) via the `pending` 1-slot box."""
    units = []
    ncol = 4 * j + 4
    for hp in range(HL // 2):
        state = {}

        def alloc(state=state):
            state["yps"] = P.y_ps.tile(
                [128, 2, 512], F32, tag="yps", name="yps", bufs=1
            )

        def sc(c, hp=hp, state=state):
            def emit():
                d = c - 4 * j  # >= 0 on the diagonal band
                off = max(d, 0) * 128  # columns below off are fully masked
                sps2 = P.attn_ps.tile(
                    [128, 2, 512], F32, tag="sps2", name="sps2"
                )
                for hi in range(2):
                    h = 2 * hp + hi
                    po = (h % 2) * 64
                    nc.tensor.matmul(
                        sps2[:, hi, off:512],
                        P.qkT[po : po + 64, 4 + h // 2, c * 128 : (c + 1) * 128],
                        P.qkT[po : po + 64, h // 2, j * 512 + off : (j + 1) * 512],
                        start=True,
                        stop=True,
                    )
                probs2 = P.probs_p.tile(
                    [128, 2, 512], BF16, tag="probs", name="probs"
                )
                nc.scalar.activation(
                    out=probs2[:, :, off:512],
                    in_=sps2[:, :, off:512],
                    func=mybir.ActivationFunctionType.Exp,
                    scale=0.125,
                )
                if d >= 0:
                    for hi in range(2):
                        nc.vector.tensor_mul(
                            probs2[:, hi, off : off + 128],
                            probs2[:, hi, off : off + 128],
                            P.mask_sb,
                        )
                state[c] = (probs2, off)

            return emit

        def av(c, hp=hp, state=state):
            def emit():
                probs2, off = state.pop(c)
                yps = state["yps"]
                for hi in range(2):
                    h = 2 * hp + hi
                    nc.tensor.matmul(
                        yps[0:VW, hi, off:512],
                        P.v_sb[:, c, h, :],
                        probs2[:, hi, off:512],
                        start=(c == 0),
                        stop=(c == ncol - 1),
                    )

            return emit

        def epilogue(hp=hp, state=state, j=j):
            # ones-row out of PSUM to partition 0 (ACT can shift
            # partitions), DVE fast reciprocal, gpsimd partition broadcast,
            # DVE scale into bf16 dim-major yT
            yps = state["yps"]
            dsb = P.den_p.tile([1, 2, 512], F32, tag="den", name="den")
            nc.scalar.activation(
                out=dsb,
                in_=yps[HD : HD + 1, :, :],
                func=mybir.ActivationFunctionType.Copy,
            )
            nc.vector.reciprocal_approx_fast(out=dsb, in_=dsb)
            denb = P.den_p.tile([HD, 2, 512], F32, tag="denb", name="denb")
            nc.gpsimd.partition_broadcast(denb, dsb)
            for hi in range(2):
                h = 2 * hp + hi
                po = (h % 2) * 64
                P.anchor = nc.vector.tensor_mul(
                    P.yT[po : po + 64, h // 2, j * 512 : (j + 1) * 512],
                    yps[0:HD, hi, :],
                    denb[:, hi, :],
                )

        units.append(alloc)
        units.append(sc(0))
        units.append(sc(1))
        if pending[0] is not None:
            units.append(pending[0])
            pending[0] = None
        units.append(av(0))
        for c in range(2, ncol):
            units.append(sc(c))
            units.append(av(c - 1))
        units.append(av(ncol - 1))
        pending[0] = epilogue
    return units


def _proj_units(nc, P, j):
    """bf16 projection partials for q-tile j -> cc_in rows, 2 units/tile."""
    units = []
    for mt in range(4 * j, 4 * j + 4):
        state = {}

        def half(nh, mt=mt, state=state):
            def emit():
                if nh == 0:
                    state["osb"] = P.out_p.tile(
                        [128, D], BF16, tag="osb", name="osb"
                    )
                osb = state["osb"]
                ps = P.b1_ps.tile([128, 512], F32, tag="b1", name="ops")
                for kk in range(GD // 128):
                    nc.tensor.matmul(
                        ps,
                        P.yT[:, kk, mt * 128 : (mt + 1) * 128],
                        P.wp_sb[:, kk, nh * 512 : (nh + 1) * 512],
                        start=(kk == 0),
                        stop=(kk == GD // 128 - 1),
                    )
                nc.vector.tensor_copy(
                    out=osb[:, nh * 512 : (nh + 1) * 512], in_=ps
                )
                if nh == 1:
                    nc.sync.dma_start(
                        out=P.cc_in[mt * 128 : (mt + 1) * 128, :], in_=osb
                    )

            return emit

        units.append(half(0))
        units.append(half(1))
    return units


def _rs_unit(nc, P, k):
    """ReduceScatter of chunk k; my half-size shard -> cc_red."""

    def emit():
        lo, n = CHUNKS[k]
        nc.gpsimd.collective_compute(
            "ReduceScatter",
            mybir.AluOpType.add,
            replica_groups=RG,
            ins=[P.cc_in[lo : lo + n, :].opt()],
            outs=[P.cc_red[lo // 2 : lo // 2 + n // 2, :].opt()],
        )

    return emit


def _cast_units(nc, P, k):
    """Upcast my bf16 shard of chunk k to fp32 and DMA it out. Anchored
    after P.anchor so the compile-time scheduler (whose cost model is
    optimistic about collectives) cannot hoist the loads into the
    attention region, where a too-early wait stalls the whole engine."""
    lo, n = CHUNKS[k]
    units = []
    for t in range(n // 256):
        def emit(t=t):
            row = lo // 2 + t * 128
            ci = P.cast_p.tile([128, D], BF16, tag="ci", name="ci")
            ld = nc.sync.dma_start(out=ci, in_=P.cc_red[row : row + 128, :])
            if P.anchor is not None:
                add_dep_helper(
                    ld.ins, P.anchor.ins, sync=True, reason="casts stay in tail"
                )
            co = P.cast_p.tile([128, D], F32, tag="co", name="co")
            nc.vector.tensor_copy(out=co, in_=ci)
            nc.sync.dma_start(out=P.out[row : row + 128, :], in_=co)

        units.append(emit)
    return units


def _zip(a_units, fill_units):
    """Spread fill_units evenly between a_units (fills go to the PE's idle
    slots while ACT drains the exps)."""
    na, nf = len(a_units), len(fill_units)
    if nf == 0:
        return list(a_units)
    pos = [int(na * (f + 1) / (nf + 1)) for f in range(nf)]
    out = []
    fi = 0
    for i, u in enumerate(a_units):
        out.append(u)
        while fi < nf and pos[fi] <= i:
            out.append(fill_units[fi])
            fi += 1
    out.extend(fill_units[fi:])
    return out


class _Ctx:
    pass


def _build_nc():
    nc = bacc.Bacc(None, num_devices=8)
    P = _Ctx()

    xTd = nc.dram_tensor("xT", [D, S], BF16, kind="ExternalInput").ap()
    wqkv = nc.dram_tensor("wqkv", [D, 3 * GD], BF16, kind="ExternalInput").ap()
    wproj = nc.dram_tensor("wproj", [GD, D], BF16, kind="ExternalInput").ap()
    masks = nc.dram_tensor("masks", [128, 128], BF16, kind="ExternalInput").ap()
    P.out = nc.dram_tensor("out", [S // 2, D], F32, kind="ExternalOutput").ap()

    with tile.TileContext(nc) as tc:
        with (
            tc.tile_pool(name="const", bufs=1) as const,
            tc.tile_pool(name="w_p", bufs=1) as w_p,
            tc.tile_pool(name="big_p", bufs=1) as big_p,
            tc.tile_pool(name="probs_p", bufs=8) as probs_p,
            tc.tile_pool(name="den_p", bufs=2) as den_p,
            tc.tile_pool(name="out_p", bufs=2) as out_p,
            tc.tile_pool(name="cast_p", bufs=2) as cast_p,
            tc.tile_pool(name="b1_ps", bufs=2, space="PSUM") as b1_ps,
            tc.tile_pool(name="attn_ps", bufs=2, space="PSUM") as attn_ps,
            tc.tile_pool(name="y_ps", bufs=1, space="PSUM") as y_ps,
            tc.tile_pool(name="dram", bufs=1, space="DRAM") as dram,
        ):
            P.probs_p, P.den_p, P.out_p, P.cast_p = probs_p, den_p, out_p, cast_p
            P.b1_ps, P.attn_ps, P.y_ps = b1_ps, attn_ps, y_ps

            # Startup: plain DMAs only, k-interleaved so QKV chain m=0 can
            # start its k-th accumulation step as soon as slice k lands.
            P.xT = big_p.tile([128, NDM, S], BF16, name="xT")
            P.w_sb = w_p.tile([128, NDM, 3 * GD], BF16, name="w_sb")
            P.mask_sb = const.tile([128, 128], BF16, name="mask_sb")
            nc.sync.dma_start(out=P.mask_sb, in_=masks)
            for k in range(NDM):
                nc.sync.dma_start(
                    out=P.xT[:, k, 0:512], in_=xTd[k * 128 : (k + 1) * 128, 0:512]
                )
                nc.sync.dma_start(
                    out=P.w_sb[:, k, 0:GD],
                    in_=wqkv[k * 128 : (k + 1) * 128, 0:GD],
                )
            # preload the exp table while DMAs run
            aw = const.tile([1, 2], F32, name="actwarm")
            nc.vector.memset(aw, 0.0)
            nc.scalar.activation(
                out=aw, in_=aw, func=mybir.ActivationFunctionType.Exp
            )
            for k in range(NDM):  # K columns
                nc.sync.dma_start(
                    out=P.w_sb[:, k, GD : 2 * GD],
                    in_=wqkv[k * 128 : (k + 1) * 128, GD : 2 * GD],
                )
            for k in range(NDM):  # V columns
                nc.sync.dma_start(
                    out=P.w_sb[:, k, 2 * GD : 3 * GD],
                    in_=wqkv[k * 128 : (k + 1) * 128, 2 * GD : 3 * GD],
                )
            P.wp_sb = w_p.tile([128, GD // 128, D], BF16, name="wp_sb")
            for kk in range(GD // 128):
                nc.sync.dma_start(
                    out=P.wp_sb[:, kk, :], in_=wproj[kk * 128 : (kk + 1) * 128, :]
                )
            for k in range(NDM):
                nc.sync.dma_start(
                    out=P.xT[:, k, 512:S], in_=xTd[k * 128 : (k + 1) * 128, 512:S]
                )

            P.qkT = big_p.tile([128, 2 * GD // 128, S], BF16, name="qkT")
            P.v_sb = big_p.tile([128, NTOK, HL, VW], BF16, name="v_sb")
            nc.vector.memset(P.v_sb[:, :, :, HD : HD + 1], 1.0)
            P.yT = big_p.tile([128, GD // 128, S], BF16, name="yT")

            P.cc_in = dram.tile([S, D], BF16, name="cc_in")
            P.cc_red = dram.tile([S // 2, D], BF16, name="cc_red")

            # QKV tile 0: only what attention (j0, hp0) needs up front; the
            # remaining chains become fill for the j0 attention region.
            q0 = _qkv_units(nc, P, 0)
            q0_lead = [q0[0], q0[4], q0[8], q0[9], q0[10], q0[11]]
            q0_rest = [q0[1], q0[5], q0[2], q0[6], q0[3], q0[7]]
            for u in q0_lead:
                u()

            pending = [None]
            P.anchor = None
            fills_by_tile = {
                0: lambda: q0_rest + _qkv_units(nc, P, 1),
                1: lambda: _qkv_units(nc, P, 2),
                2: lambda: (
                    _qkv_units(nc, P, 3)
                    + _proj_units(nc, P, 0)
                    + [_rs_unit(nc, P, 0)]
                ),
                3: lambda: (
                    _proj_units(nc, P, 1)
                    + [_rs_unit(nc, P, 1)]
                    + _proj_units(nc, P, 2)
                    + [_rs_unit(nc, P, 2)]
                ),
            }
            for j in range(NQT):
                a_units = _attn_units(nc, P, j, pending)
                for u in _zip(a_units, fills_by_tile[j]()):
                    u()

            # tail: final epilogue (sets the cast anchor), last q-tile's
            # projection, final ReduceScatter, then all casts (chunks 0-2
            # are long since reduced; their upcast+out-DMA hides under the
            # final collective)
            pending[0]()
            pending[0] = None
            for u in _proj_units(nc, P, 3):
                u()
            _rs_unit(nc, P, 3)()
            for k in range(4):
                for cu in _cast_units(nc, P, k):
                    cu()

    nc.compile()
    return nc


def _host_consts():
    ki = np.arange(128)[:, None]
    qj = np.arange(128)[None, :]
    masks = (qj >= ki).astype(ml_dtypes.bfloat16)  # [128, 128] diagonal band
    return masks


def _in_maps(x, w_qkv, w_proj):
    masks = _host_consts()
    xT = {}
    wq16 = {}
    wp16 = {}
    maps = []
    for c in range(8):
        b, g = c // 2, c % 2
        if b not in xT:
            xT[b] = np.ascontiguousarray(x[b].T).astype(ml_dtypes.bfloat16)
        if g not in wq16:
            wq = w_qkv[:, g * GD : (g + 1) * GD]
            wk = w_qkv[:, D + g * GD : D + (g + 1) * GD]
            wv = w_qkv[:, 2 * D + g * GD : 2 * D + (g + 1) * GD]
            wq16[g] = np.ascontiguousarray(
                np.concatenate([wq, wk, wv], axis=1)
            ).astype(ml_dtypes.bfloat16)
            wp16[g] = np.ascontiguousarray(
                w_proj[g * GD : (g + 1) * GD, :]
            ).astype(ml_dtypes.bfloat16)
        maps.append(
            {"xT": xT[b], "wqkv": wq16[g], "wproj": wp16[g], "masks": masks}
        )
    return maps


def kernel(x, w_qkv, w_proj):
    x = np.ascontiguousarray(x, dtype=np.float32)
    w_qkv = np.ascontiguousarray(w_qkv, dtype=np.float32)
    w_proj = np.ascontiguousarray(w_proj, dtype=np.float32)
    if "nc" not in _NC_CACHE:
        _NC_CACHE["nc"] = _build_nc()
    nc = _NC_CACHE["nc"]
    r = run_bass_kernel_spmd(nc, _in_maps(x, w_qkv, w_proj), list(range(8)))
    out = np.empty((4, S, D), np.float32)
    for b in range(4):
        for rk in range(2):
            o = r.results[2 * b + rk]["out"]
            for k, (lo, n) in enumerate(CHUNKS):
                h = n // 2
                out[b, lo + rk * h : lo + (rk + 1) * h] = o[
                    lo // 2 : lo // 2 + h
                ]
    return out
